# revision 4
# baseline (speedup 1.0000x reference)
"""Trainium2 Bass kernel for CompositionalResidualMLP (MoE routing, 2-node module network).

Strategy: expert-parallel over the node-1 module (a1): core c receives exactly the
samples with a1==c, so L3/L4/L5 are single-module dense matmuls with large merged
PSUM evictions, and only module c's node-1 weights are shipped to core c. Columns
are grouped into 8 contiguous a0-blocks (capacity C each) so L1/L2 are dense
per-module matmuls; modules i and i+4 run concurrently on PE row-halves.
All weights+biases ship as one packed bf16 image (2 DMA issues); x0/x1 in 2 chunks
each; output DMAs issue from the GpSimd queue so the Sync queue only handles input.
PSUM evictions (fp32 -> bf16 with fused bias+ReLU) are explicitly balanced across
the Scalar and Vector engines, which are the throughput bound for this problem.
"""

import numpy as np
from contextlib import ExitStack

# Problem constants (hardcoded per contract)
B_TOT = 32768
D0 = 64
D1 = 64
M = 8          # modules per node
H = 256        # hidden width
O0 = 128
O1 = 32
NCORES = 8

WARMUP_MMS = 8   # bf16 warm-up matmuls at kernel start (HAM clock-gate warm)

# packed weight-image column offsets (bf16 cols)
W00_OFF = 0            # [128, 1024]  (modules 0-3 rows 0-63, 4-7 rows 64-127)
BIAS_OFF = 1024        # [128, 56]    (28 fp32 cols bitcast)
W1P_OFF = 1080         # [128, 256]   (module c, rows duplicated on halves)
W01_OFF = 1336         # [128, 2048]  (kc-major, per-module 128-col chunks)
W1A_OFF = 3384         # [128, 384]   (module c, 3 K-chunks)
W1O_OFF = 3768         # [128, 32]    (module c)
WB_COLS = 3800
WA_END = W01_OFF       # first weight DMA covers w00+biases+w1p


def _build_bass(C, ncol):
    """Per-core Bass program; identical on all cores (pure SPMD)."""
    import concourse.bass as bass
    import concourse.tile as tile
    from concourse import bacc, mybir

    F32 = mybir.dt.float32
    BF16 = mybir.dt.bfloat16
    Relu = mybir.ActivationFunctionType.Relu
    Identity = mybir.ActivationFunctionType.Identity
    Add = mybir.AluOpType.add
    Max = mybir.AluOpType.max

    ch = C // 2            # PSUM chunk size (<=512 fp32 per bank)
    nch = 2 * M            # 16 chunks of ch cols each
    half = ncol // 2       # = 4*C

    nc = bacc.Bacc("TRN2", target_bir_lowering=False, debug=False,
                   enable_asserts=False, num_devices=NCORES)

    x0t = nc.dram_tensor("x0t", [128, half], BF16, kind="ExternalInput").ap()
    x1t = nc.dram_tensor("x1t", [128, half], BF16, kind="ExternalInput").ap()
    wb = nc.dram_tensor("wb", [128, WB_COLS], BF16, kind="ExternalInput").ap()
    outT = nc.dram_tensor("outT", [128, 4 * ch], F32, kind="ExternalOutput").ap()

    with tile.TileContext(nc) as tc:
        with ExitStack() as ctx:
            acts = ctx.enter_context(tc.tile_pool(name="acts", bufs=1))
            ps = ctx.enter_context(tc.tile_pool(name="ps", bufs=2, space="PSUM"))

            wbs = acts.tile([128, WB_COLS], BF16, tag="wb")
            x0s = acts.tile([128, half], BF16, tag="x0")
            x1s = acts.tile([128, half], BF16, tag="x1")
            h1 = acts.tile([128, 2 * ncol], BF16, tag="h1")   # mo-major
            g1 = acts.tile([128, 2 * ncol], BF16, tag="g1")   # mo-major
            hs = acts.tile([128, ncol], BF16, tag="h")
            gs = acts.tile([128, ncol], BF16, tag="g")
            outs = acts.tile([128, 4 * ch], F32, tag="out")
            wu = acts.tile([128, 512], BF16, tag="wu")

            # input DMAs on the Sync queue, ordered by consumption
            nc.sync.dma_start(wbs[:, 0:WA_END], wb[:, 0:WA_END])
            nc.sync.dma_start(x0s[:, 0:2 * C], x0t[:, 0:2 * C])
            nc.sync.dma_start(x1s[:, 0:2 * C], x1t[:, 0:2 * C])
            nc.sync.dma_start(x0s[:, 2 * C:half], x0t[:, 2 * C:half])
            nc.sync.dma_start(x1s[:, 2 * C:half], x1t[:, 2 * C:half])
            nc.sync.dma_start(wbs[:, WA_END:WB_COLS], wb[:, WA_END:WB_COLS])

            # HAM warm-up burst while inputs stream in
            nc.vector.memset(wu[:], 0.0)
            for _ in range(WARMUP_MMS):
                pw = ps.tile([128, 2048], F32, tag="ps")
                nc.tensor.matmul(pw[:, 0:512], wu[:, 0:128], wu[:],
                                 start=True, stop=True)

            bias = wbs[:, BIAS_OFF:BIAS_OFF + 56].bitcast(F32)  # [128, 28] fp32

            def bias_ap(j):
                return bias[:, j:j + 1]

            evict_parity = [0]

            def evict(dst_ap, src_ap, bap, relu, force=None):
                """PSUM->SBUF with fused bias+activation, alternating engines."""
                p = evict_parity[0] if force is None else force
                if force is None:
                    evict_parity[0] ^= 1
                if p == 0:
                    nc.scalar.activation(dst_ap, src_ap, Relu if relu else Identity,
                                         bias=bap)
                else:
                    if relu:
                        nc.vector.tensor_scalar(dst_ap, src_ap, bap, 0.0, Add, Max)
                    else:
                        nc.vector.tensor_scalar_add(dst_ap, src_ap, bap)

            def bank(pt, b, n=None):
                return pt[:, b * 512: b * 512 + (ch if n is None else n)]

            def banks2(pt, b0):
                # [128, 2, ch] view over banks b0, b0+1
                v = pt[:].rearrange("p (b c) -> p b c", b=4)
                return v[:, b0:b0 + 2, 0:ch]

            def banks4(pt):
                v = pt[:].rearrange("p (b c) -> p b c", b=4)
                return v[:, :, 0:ch]

            # ---- L1: h1[mo] = relu(W00[a0][:,mo]^T x0 + b00), K=64 row-half packed
            # ---- L3: g1[mo] = relu(W1p[c][:,mo]^T x1 + b1p), K=64 row-half packed
            # interleaved so L3's x1 DMA arrival overlaps L1 compute
            def l1_tile(ip, mo):
                pt = ps.tile([128, 2048], F32, tag="ps")
                for blk, rows in ((ip, slice(0, 64)), (ip + 4, slice(64, 128))):
                    b0 = 0 if blk == ip else 2
                    w = wbs[rows, W00_OFF + ip * 256 + mo * 128:
                            W00_OFF + ip * 256 + mo * 128 + 128]
                    for c in range(2):
                        nc.tensor.matmul(
                            bank(pt, b0 + c),
                            w, x0s[rows, blk % 4 * C + c * ch: blk % 4 * C + (c + 1) * ch],
                            start=True, stop=True)
                for blk, b0 in ((ip, 0), (ip + 4, 2)):
                    dst = h1[:, mo * ncol + blk * C: mo * ncol + (blk + 1) * C]
                    evict(dst.rearrange("p (b c) -> p b c", b=2),
                          banks2(pt, b0), bias_ap(mo * 8 + blk), True)

            def l3_tile(k, mo):
                # chunks 2k, 2k+1 on both row-halves (lo: blocks 0-3, hi: 4-7)
                pt = ps.tile([128, 2048], F32, tag="ps")
                for j in range(2):
                    cc = (2 * k + j) * ch
                    for rows, b0 in ((slice(0, 64), 0), (slice(64, 128), 2)):
                        w = wbs[rows, W1P_OFF + mo * 128: W1P_OFF + mo * 128 + 128]
                        nc.tensor.matmul(bank(pt, b0 + j), w,
                                         x1s[rows, cc: cc + ch],
                                         start=True, stop=True)
                # dst: lo chunks at cols [2k*ch ...), hi chunks at half + [2k*ch ...)
                dlo = g1[:, mo * ncol + 2 * k * ch: mo * ncol + (2 * k + 2) * ch]
                dhi = g1[:, mo * ncol + half + 2 * k * ch:
                         mo * ncol + half + (2 * k + 2) * ch]
                src = pt[:].rearrange("p (b c) -> p b c", b=4)
                evict(dlo.rearrange("p (b c) -> p b c", b=2),
                      src[:, 0:2, 0:ch], bias_ap(24 + mo), True, force=k % 2)
                evict(dhi.rearrange("p (b c) -> p b c", b=2),
                      src[:, 2:4, 0:ch], bias_ap(24 + mo), True, force=(k + 1) % 2)

            order = []
            for ip in range(4):
                order.append(("l1", ip, 0))
                order.append(("l1", ip, 1))
                if ip % 2 == 1:
                    mo = ip // 2
                    order.append(("l3", 0, mo))
                    order.append(("l3", 1, mo))
                    order.append(("l3", 2, mo))
                    order.append(("l3", 3, mo))
            # reorder: emit l3 tiles for mo over both mo values after all l1
            for kind, a, b in order:
                if kind == "l1":
                    l1_tile(a, b)
                else:
                    l3_tile(a, b)

            # ---- L2: h = relu(W01[a0]^T h1 + b01), K=256 (2 accum chunks)
            for t in range(4):           # blocks 2t, 2t+1
                pt = ps.tile([128, 2048], F32, tag="ps")
                for j, blk in enumerate((2 * t, 2 * t + 1)):
                    for c in range(2):
                        for kc in range(2):
                            w = wbs[:, W01_OFF + kc * 1024 + blk * 128:
                                    W01_OFF + kc * 1024 + blk * 128 + 128]
                            nc.tensor.matmul(
                                bank(pt, 2 * j + c), w,
                                h1[:, kc * ncol + blk * C + c * ch:
                                   kc * ncol + blk * C + (c + 1) * ch],
                                start=(kc == 0), stop=(kc == 1))
                for j, blk in enumerate((2 * t, 2 * t + 1)):
                    dst = hs[:, blk * C: (blk + 1) * C]
                    evict(dst.rearrange("p (b c) -> p b c", b=2),
                          banks2(pt, 2 * j), bias_ap(16 + blk), True)

            # ---- L4: g = relu(W1a[c]^T [h; g1_mo0; g1_mo1] + b1a), K=384
            for t in range(4):           # chunks 4t .. 4t+3
                pt = ps.tile([128, 2048], F32, tag="ps")
                for j in range(4):
                    cc = (4 * t + j) * ch
                    for kc in range(3):
                        if kc == 0:
                            rhs = hs[:, cc: cc + ch]
                        else:
                            rhs = g1[:, (kc - 1) * ncol + cc: (kc - 1) * ncol + cc + ch]
                        w = wbs[:, W1A_OFF + kc * 128: W1A_OFF + kc * 128 + 128]
                        nc.tensor.matmul(bank(pt, j), w, rhs,
                                         start=(kc == 0), stop=(kc == 2))
                dst = gs[:, 4 * t * ch: (4 * t + 4) * ch]
                evict(dst.rearrange("p (b c) -> p b c", b=4),
                      banks4(pt), bias_ap(26), True, force=t % 2)

            # ---- L5: out = W1o[c]^T g + b1o (identity), col-tiled 4x32
            w1o = wbs[:, W1O_OFF: W1O_OFF + 32]
            for t in range(2):           # slots 2t, 2t+1
                pt = ps.tile([128, 2048], F32, tag="ps")
                for j, s in enumerate((2 * t, 2 * t + 1)):
                    for g in range(4):
                        cc = (4 * s + g) * ch
                        nc.tensor.matmul(
                            pt[32 * g: 32 * g + 32, j * 512: j * 512 + ch],
                            w1o, gs[:, cc: cc + ch],
                            start=True, stop=True, tile_position=(0, 32 * g))
                dst = outs[:, 2 * t * ch: (2 * t + 2) * ch]
                evict(dst.rearrange("p (b c) -> p b c", b=2),
                      banks2(pt, 0)[:, :, :], bias_ap(27), False, force=t)
                nc.gpsimd.dma_start(outT[:, 2 * t * ch: (2 * t + 2) * ch],
                                    outs[:, 2 * t * ch: (2 * t + 2) * ch])

    nc.compile()
    return nc


def _route(input_val):
    """core = a1; columns grouped into 8 a0-blocks of capacity C per core."""
    a0 = np.argmax(input_val[:, D0 + D1: D0 + D1 + M], axis=1)
    a1 = np.argmax(input_val[:, D0 + D1 + M: D0 + D1 + 2 * M], axis=1)
    B = input_val.shape[0]
    nij = np.zeros((M, M), dtype=np.int64)
    np.add.at(nij, (a1, a0), 1)          # [core, block]
    C = int(-(-nij.max() // 4) * 4)      # capacity, multiple of 4
    ncol = M * C

    key = a1 * M + a0
    order = np.argsort(key, kind="stable")
    counts = np.bincount(key, minlength=M * M)
    group_start = np.concatenate([[0], np.cumsum(counts)[:-1]])
    rank_sorted = np.arange(B) - np.repeat(group_start, counts)
    rank = np.empty(B, dtype=np.int64)
    rank[order] = rank_sorted
    core = a1
    col = a0 * C + rank
    assert np.all(rank < C), "capacity overflow"
    return core, col, a0, C, ncol


def _pack(inputs, core_of, C, ncol):
    import ml_dtypes
    bf = ml_dtypes.bfloat16
    W00 = inputs["W00"].astype(np.float32)
    W01 = inputs["W01"].astype(np.float32)
    W1p = inputs["W1p"].astype(np.float32)
    W1a = inputs["W1a"].astype(np.float32)
    W1o = inputs["W1o"].astype(np.float32)
    b00 = inputs["b00"].astype(np.float32)
    b01 = inputs["b01"].astype(np.float32)
    b1p = inputs["b1p"].astype(np.float32)
    b1a = inputs["b1a"].astype(np.float32)
    b1o = inputs["b1o"].astype(np.float32)

    wbs = []
    for c in range(NCORES):
        img = np.zeros((128, WB_COLS), dtype=bf)
        img[0:64, 0:1024] = W00[:4].transpose(1, 0, 2).reshape(64, 1024).astype(bf)
        img[64:128, 0:1024] = W00[4:].transpose(1, 0, 2).reshape(64, 1024).astype(bf)
        bias = np.zeros((128, 28), dtype=np.float32)
        # b00: col mo*8+i ; b01: col 16+i ; b1p: col 24+mo ; b1a: 26 ; b1o: 27
        bias[:, 0:16] = b00.reshape(M, 2, 128).transpose(2, 1, 0).reshape(128, 16)
        bias[:, 16:24] = b01.T
        bias[:, 24:26] = b1p[c].reshape(2, 128).T
        bias[:, 26] = b1a[c]
        bias[:, 27] = np.tile(b1o[c], 4)
        img[:, BIAS_OFF:BIAS_OFF + 56] = bias.view(np.uint32).view(np.uint16).reshape(
            128, 56).view(bf)
        wp = W1p[c].astype(bf)                       # [64, 256]
        img[0:64, W1P_OFF:W1P_OFF + 256] = wp
        img[64:128, W1P_OFF:W1P_OFF + 256] = wp
        img[:, W01_OFF:W01_OFF + 2048] = W01.reshape(M, 2, 128, O0).transpose(
            2, 1, 0, 3).reshape(128, 2048).astype(bf)
        img[:, W1A_OFF:W1A_OFF + 384] = W1a[c].reshape(3, 128, O0).transpose(
            1, 0, 2).reshape(128, 384).astype(bf)
        img[:, W1O_OFF:W1O_OFF + 32] = W1o[c].astype(bf)
        wbs.append(np.ascontiguousarray(img))
    return wbs


def kernel(**inputs):
    import os
    import ml_dtypes
    from concourse.bass_utils import run_bass_kernel_spmd

    bf = ml_dtypes.bfloat16
    input_val = np.asarray(inputs["input_val"], dtype=np.float32)
    B = input_val.shape[0]

    core, col, a0, C, ncol = _route(input_val)
    half = ncol // 2
    ch = C // 2

    feat0 = input_val[:, :D0].astype(bf)
    feat1 = input_val[:, D0:D0 + D1].astype(bf)
    X0T = np.zeros((NCORES, 128, half), dtype=bf)
    X1T = np.zeros((NCORES, 128, half), dtype=bf)
    hi = a0 >= 4
    prow = np.where(hi, 64, 0)
    pcol = np.where(hi, col - half, col)
    for r in (0, 64):
        m = prow == r
        X0T[core[m], r:r + 64, pcol[m]] = feat0[m]
        X1T[core[m], r:r + 64, pcol[m]] = feat1[m]

    wbs = _pack(inputs, core, C, ncol)
    nc = _build_bass(C, ncol)

    in_maps = [dict(wb=wbs[c], x0t=np.ascontiguousarray(X0T[c]),
                    x1t=np.ascontiguousarray(X1T[c])) for c in range(NCORES)]
    res = run_bass_kernel_spmd(nc, in_maps, core_ids=list(range(NCORES)),
                               tmpdir=os.environ.get("BASS_TMPDIR"))
    global _LAST_RESULTS
    _LAST_RESULTS = res

    OUT = np.stack([r["outT"] for r in res.results])  # [NCORES, 128, 4*ch]
    # decode: sample at core-col q sits at chunk k=q//ch, g=k%4, s=k//4:
    #   row 32*g+o, col s*ch + q%ch
    k = col // ch
    cc = col % ch
    rows = (32 * (k % 4))[:, None] + np.arange(O1)[None, :]
    cols = ((k // 4) * ch + cc)[:, None]
    out = OUT[core[:, None], rows, cols]
    return np.ascontiguousarray(out).astype(np.float32)


# revision 5
# speedup vs baseline: 1.1392x; 1.1392x over previous
"""Trainium2 Bass kernel for CompositionalResidualMLP (MoE routing, 2-node module network).

Strategy: expert-parallel over the node-1 module (a1): core c receives exactly the
samples with a1==c, so L3/L4/L5 are single-module dense matmuls with large merged
PSUM evictions, and only module c's node-1 weights ship to core c. Columns are
grouped into 8 contiguous a0-blocks (capacity C) so L1/L2 are dense per-module
matmuls; modules i and i+4 run concurrently on PE row-halves. PSUM is managed as
one 8-bank tile with a manual 4-slot rotation so matmul fill, Scalar-engine
eviction and Vector-engine eviction all overlap; eviction atoms are assigned to
the two engines by a measured-cost greedy balance. A 12-matmul warm-up burst
keeps the PE clock gate at 8/8 through the DMA lead-in; inputs arrive in 7 issues
ordered by first use; outputs stream back from the GpSimd queue per L5 chunk.
"""

import numpy as np
from contextlib import ExitStack

B_TOT = 32768
D0 = 64
D1 = 64
M = 8
H = 256
O0 = 128
O1 = 32
NCORES = 8

WARMUP_MMS = 12

W00_OFF = 0            # [128, 1024]
BIAS_OFF = 1024        # [128, 56]  (28 fp32 cols bitcast)
W1P_OFF = 1080         # [128, 256]
W01_OFF = 1336         # [128, 2048]
W1A_OFF = 3384         # [128, 384]
W1O_OFF = 3768         # [128, 32]
WB_COLS = 3800


def _build_bass(C, ncol):
    import concourse.bass as bass
    import concourse.tile as tile
    from concourse import bacc, mybir

    F32 = mybir.dt.float32
    BF16 = mybir.dt.bfloat16
    Relu = mybir.ActivationFunctionType.Relu
    Identity = mybir.ActivationFunctionType.Identity
    Add = mybir.AluOpType.add
    Max = mybir.AluOpType.max

    ch = C // 2
    half = ncol // 2

    nc = bacc.Bacc("TRN2", target_bir_lowering=False, debug=False,
                   enable_asserts=False, num_devices=NCORES)

    x0t = nc.dram_tensor("x0t", [128, half], BF16, kind="ExternalInput").ap()
    x1t = nc.dram_tensor("x1t", [128, half], BF16, kind="ExternalInput").ap()
    wb = nc.dram_tensor("wb", [128, WB_COLS], BF16, kind="ExternalInput").ap()
    outT = nc.dram_tensor("outT", [128, 4 * ch], F32, kind="ExternalOutput").ap()

    with tile.TileContext(nc) as tc:
        with ExitStack() as ctx:
            acts = ctx.enter_context(tc.tile_pool(name="acts", bufs=1))
            ps = ctx.enter_context(tc.tile_pool(name="ps", bufs=1, space="PSUM"))

            wbs = acts.tile([128, WB_COLS], BF16, tag="wb")
            x0s = acts.tile([128, half], BF16, tag="x0")
            x1s = acts.tile([128, half], BF16, tag="x1")
            h1 = acts.tile([128, 2 * ncol], BF16, tag="h1")   # mo-major
            g1 = acts.tile([128, 2 * ncol], BF16, tag="g1")   # mo-major
            hs = acts.tile([128, ncol], BF16, tag="h")
            gs = acts.tile([128, ncol], BF16, tag="g")
            outs = acts.tile([128, 4 * ch], F32, tag="out")
            wu = acts.tile([128, 512], BF16, tag="wu")
            psall = ps.tile([128, 4096], F32, tag="ps")       # all 8 PSUM banks

            # input DMAs ordered by first consumption (Sync queue)
            nc.sync.dma_start(wbs[:, 0:W1P_OFF], wb[:, 0:W1P_OFF])
            nc.sync.dma_start(x0s[:, 0:2 * C], x0t[:, 0:2 * C])
            nc.sync.dma_start(x1s[:, 0:2 * C], x1t[:, 0:2 * C])
            nc.sync.dma_start(wbs[:, W1P_OFF:W01_OFF], wb[:, W1P_OFF:W01_OFF])
            nc.sync.dma_start(x0s[:, 2 * C:half], x0t[:, 2 * C:half])
            nc.sync.dma_start(x1s[:, 2 * C:half], x1t[:, 2 * C:half])
            nc.sync.dma_start(wbs[:, W01_OFF:WB_COLS], wb[:, W01_OFF:WB_COLS])

            slot_ctr = [0]

            def alloc(n):
                """n slots of 2 banks each (n in {1,2}); returns [128, n*1024] AP."""
                s = slot_ctr[0]
                if n == 2 and s % 2 == 1:
                    s += 1
                r = s % 4
                slot_ctr[0] = s + n
                return psall[:, r * 1024: (r + n) * 1024]

            # HAM warm-up burst (~5us sustained) while inputs stream in
            nc.vector.memset(wu[:], 0.0)
            for _ in range(WARMUP_MMS):
                pw = alloc(1)
                nc.tensor.matmul(pw[:, 0:512], wu[:, 0:128], wu[:],
                                 start=True, stop=True)

            bias = wbs[:, BIAS_OFF:BIAS_OFF + 56].bitcast(F32)  # [128, 28]

            def bias_ap(j):
                return bias[:, j:j + 1]

            load = [0.0, 0.0]
            COST = {(0, 1): 840.0, (0, 2): 1317.0, (1, 1): 932.0, (1, 2): 1528.0}

            def evict(dst_ap, src_ap, bap, relu, nslots):
                e = 0 if load[0] + COST[(0, nslots)] <= load[1] + COST[(1, nslots)] \
                    else 1
                load[e] += COST[(e, nslots)]
                if e == 0:
                    nc.scalar.activation(dst_ap, src_ap, Relu if relu else Identity,
                                         bias=bap)
                else:
                    if relu:
                        nc.vector.tensor_scalar(dst_ap, src_ap, bap, 0.0, Add, Max)
                    else:
                        nc.vector.tensor_scalar_add(dst_ap, src_ap, bap)

            def b2(region):
                return region[:].rearrange("p (b c) -> p b c", b=2)[:, :, 0:ch]

            def b4(region):
                return region[:].rearrange("p (b c) -> p b c", b=4)[:, :, 0:ch]

            # ---- L1 unit (blk, mo): 1 slot, K=64 on a row-half
            def l1_unit(blk, mo):
                rows = slice(0, 64) if blk < 4 else slice(64, 128)
                pr = alloc(1)
                w = wbs[rows, (blk % 4) * 256 + mo * 128: (blk % 4) * 256 + mo * 128 + 128]
                for c in range(2):
                    nc.tensor.matmul(pr[:, c * 512: c * 512 + ch], w,
                                     x0s[rows, (blk % 4) * C + c * ch:
                                         (blk % 4) * C + (c + 1) * ch],
                                     start=True, stop=True)
                dst = h1[:, mo * ncol + blk * C: mo * ncol + (blk + 1) * C]
                evict(dst.rearrange("p (b c) -> p b c", b=2), b2(pr),
                      bias_ap(mo * 8 + blk), True, 1)

            # ---- L3 unit (k, mo): 2 slots, chunks 2k,2k+1 on both row-halves
            def l3_unit(k, mo):
                pr = alloc(2)
                for j in range(2):
                    cc = (2 * k + j) * ch
                    for rows, b0 in ((slice(0, 64), 0), (slice(64, 128), 2)):
                        w = wbs[rows, W1P_OFF + mo * 128: W1P_OFF + mo * 128 + 128]
                        nc.tensor.matmul(pr[:, (b0 + j) * 512: (b0 + j) * 512 + ch],
                                         w, x1s[rows, cc: cc + ch],
                                         start=True, stop=True)
                dst = g1[:].rearrange("p (m x) -> p m x", m=2)[:, mo] \
                    .rearrange("p (l x) -> p l x", l=2)[:, :, 2 * k * ch:
                                                        (2 * k + 2) * ch] \
                    .rearrange("p l (b c) -> p l b c", b=2)
                src = b4(pr).rearrange("p (l b) c -> p l b c", l=2)
                evict(dst, src, bias_ap(24 + mo), True, 2)

            # emission matched to DMA arrival: x0a -> x1a -> x0b -> x1b
            for ip in (0, 1):
                for mo in range(2):
                    l1_unit(ip, mo)
                    l1_unit(ip + 4, mo)
            for k in (0, 1):
                for mo in range(2):
                    l3_unit(k, mo)
            for ip in (2, 3):
                for mo in range(2):
                    l1_unit(ip, mo)
                    l1_unit(ip + 4, mo)
            for k in (2, 3):
                for mo in range(2):
                    l3_unit(k, mo)

            # ---- L2 unit (blk): 1 slot, K=256 (2 accum chunks)
            for blk in range(M):
                pr = alloc(1)
                for c in range(2):
                    for kc in range(2):
                        w = wbs[:, W01_OFF + kc * 1024 + blk * 128:
                                W01_OFF + kc * 1024 + blk * 128 + 128]
                        nc.tensor.matmul(
                            pr[:, c * 512: c * 512 + ch], w,
                            h1[:, kc * ncol + blk * C + c * ch:
                               kc * ncol + blk * C + (c + 1) * ch],
                            start=(kc == 0), stop=(kc == 1))
                dst = hs[:, blk * C: (blk + 1) * C]
                evict(dst.rearrange("p (b c) -> p b c", b=2), b2(pr),
                      bias_ap(16 + blk), True, 1)

            # ---- L4 unit (t): 2 slots, chunks 4t..4t+3, K=384 (3 accum)
            for t in range(4):
                pr = alloc(2)
                for j in range(4):
                    cc = (4 * t + j) * ch
                    for kc in range(3):
                        rhs = hs[:, cc: cc + ch] if kc == 0 else \
                            g1[:, (kc - 1) * ncol + cc: (kc - 1) * ncol + cc + ch]
                        w = wbs[:, W1A_OFF + kc * 128: W1A_OFF + kc * 128 + 128]
                        nc.tensor.matmul(pr[:, j * 512: j * 512 + ch], w, rhs,
                                         start=(kc == 0), stop=(kc == 2))
                dst = gs[:, 4 * t * ch: (4 * t + 4) * ch]
                evict(dst.rearrange("p (b c) -> p b c", b=4), b4(pr),
                      bias_ap(26), True, 2)

            # ---- L5 unit (t): 1 slot, col-tiled 4x32, identity+bias
            w1o = wbs[:, W1O_OFF: W1O_OFF + 32]
            for t in range(2):
                pr = alloc(1)
                for j, s in enumerate((2 * t, 2 * t + 1)):
                    for g in range(4):
                        cc = (4 * s + g) * ch
                        nc.tensor.matmul(
                            pr[32 * g: 32 * g + 32, j * 512: j * 512 + ch],
                            w1o, gs[:, cc: cc + ch],
                            start=True, stop=True, tile_position=(0, 32 * g))
                dst = outs[:, 2 * t * ch: (2 * t + 2) * ch]
                evict(dst.rearrange("p (b c) -> p b c", b=2), b2(pr),
                      bias_ap(27), False, 1)
                nc.gpsimd.dma_start(outT[:, 2 * t * ch: (2 * t + 2) * ch],
                                    outs[:, 2 * t * ch: (2 * t + 2) * ch])

    nc.compile()
    return nc


def _route(input_val):
    a0 = np.argmax(input_val[:, D0 + D1: D0 + D1 + M], axis=1)
    a1 = np.argmax(input_val[:, D0 + D1 + M: D0 + D1 + 2 * M], axis=1)
    B = input_val.shape[0]
    nij = np.zeros((M, M), dtype=np.int64)
    np.add.at(nij, (a1, a0), 1)
    C = int(-(-nij.max() // 4) * 4)
    ncol = M * C

    key = a1 * M + a0
    order = np.argsort(key, kind="stable")
    counts = np.bincount(key, minlength=M * M)
    group_start = np.concatenate([[0], np.cumsum(counts)[:-1]])
    rank_sorted = np.arange(B) - np.repeat(group_start, counts)
    rank = np.empty(B, dtype=np.int64)
    rank[order] = rank_sorted
    assert np.all(rank < C), "capacity overflow"
    return a1, a0 * C + rank, a0, C, ncol


def _pack(inputs):
    import ml_dtypes
    bf = ml_dtypes.bfloat16
    W00 = inputs["W00"].astype(np.float32)
    W01 = inputs["W01"].astype(np.float32)
    W1p = inputs["W1p"].astype(np.float32)
    W1a = inputs["W1a"].astype(np.float32)
    W1o = inputs["W1o"].astype(np.float32)
    b00 = inputs["b00"].astype(np.float32)
    b01 = inputs["b01"].astype(np.float32)
    b1p = inputs["b1p"].astype(np.float32)
    b1a = inputs["b1a"].astype(np.float32)
    b1o = inputs["b1o"].astype(np.float32)

    wbs = []
    for c in range(NCORES):
        img = np.zeros((128, WB_COLS), dtype=bf)
        img[0:64, 0:1024] = W00[:4].transpose(1, 0, 2).reshape(64, 1024).astype(bf)
        img[64:128, 0:1024] = W00[4:].transpose(1, 0, 2).reshape(64, 1024).astype(bf)
        bias = np.zeros((128, 28), dtype=np.float32)
        bias[:, 0:16] = b00.reshape(M, 2, 128).transpose(2, 1, 0).reshape(128, 16)
        bias[:, 16:24] = b01.T
        bias[:, 24:26] = b1p[c].reshape(2, 128).T
        bias[:, 26] = b1a[c]
        bias[:, 27] = np.tile(b1o[c], 4)
        img[:, BIAS_OFF:BIAS_OFF + 56] = bias.view(np.uint16).view(bf)
        wp = W1p[c].astype(bf)
        img[0:64, W1P_OFF:W1P_OFF + 256] = wp
        img[64:128, W1P_OFF:W1P_OFF + 256] = wp
        img[:, W01_OFF:W01_OFF + 2048] = W01.reshape(M, 2, 128, O0).transpose(
            2, 1, 0, 3).reshape(128, 2048).astype(bf)
        img[:, W1A_OFF:W1A_OFF + 384] = W1a[c].reshape(3, 128, O0).transpose(
            1, 0, 2).reshape(128, 384).astype(bf)
        img[:, W1O_OFF:W1O_OFF + 32] = W1o[c].astype(bf)
        wbs.append(np.ascontiguousarray(img))
    return wbs


def kernel(**inputs):
    import os
    import ml_dtypes
    from concourse.bass_utils import run_bass_kernel_spmd

    bf = ml_dtypes.bfloat16
    input_val = np.asarray(inputs["input_val"], dtype=np.float32)

    core, col, a0, C, ncol = _route(input_val)
    half = ncol // 2
    ch = C // 2

    feat0 = input_val[:, :D0].astype(bf)
    feat1 = input_val[:, D0:D0 + D1].astype(bf)
    X0T = np.zeros((NCORES, 128, half), dtype=bf)
    X1T = np.zeros((NCORES, 128, half), dtype=bf)
    hi = a0 >= 4
    prow = np.where(hi, 64, 0)
    pcol = np.where(hi, col - half, col)
    for r in (0, 64):
        m = prow == r
        X0T[core[m], r:r + 64, pcol[m]] = feat0[m]
        X1T[core[m], r:r + 64, pcol[m]] = feat1[m]

    wbs = _pack(inputs)
    nc = _build_bass(C, ncol)

    in_maps = [dict(wb=wbs[c], x0t=np.ascontiguousarray(X0T[c]),
                    x1t=np.ascontiguousarray(X1T[c])) for c in range(NCORES)]
    res = run_bass_kernel_spmd(nc, in_maps, core_ids=list(range(NCORES)),
                               tmpdir=os.environ.get("BASS_TMPDIR"))
    global _LAST_RESULTS
    _LAST_RESULTS = res

    OUT = np.stack([r["outT"] for r in res.results])
    k = col // ch
    cc = col % ch
    rows = (32 * (k % 4))[:, None] + np.arange(O1)[None, :]
    cols = ((k // 4) * ch + cc)[:, None]
    out = OUT[core[:, None], rows, cols]
    return np.ascontiguousarray(out).astype(np.float32)


# revision 6
# speedup vs baseline: 1.2138x; 1.0655x over previous
"""Trainium2 Bass kernel for CompositionalResidualMLP (MoE routing, 2-node module network).

Strategy: expert-parallel over the node-1 module (a1): core c receives exactly the
samples with a1==c, so L3/L4/L5 are single-module dense matmuls and only module
c's node-1 weights ship to core c. Columns are grouped into 8 contiguous
a0-blocks (capacity C) so L1/L2 are dense per-module matmuls; modules i and i+4
run concurrently on PE row-halves. All inputs ship as ONE packed bf16 image in 4
consumption-ordered DMA issues (DMA descriptor issue on the Sync queue costs
~600ns each, serialized). PSUM is one 8-bank tile under a manual 4-slot rotation;
every eviction atom is one 2-bank slot (fp32->bf16 with fused bias+ReLU), so
matmul fill and the Scalar/Vector eviction engines - the throughput bound - stay
4-deep pipelined. Atoms are balanced across ScalarE/VectorE by measured cost.
Outputs stream back from the GpSimd queue per L5 chunk.
"""

import numpy as np
from contextlib import ExitStack

B_TOT = 32768
D0 = 64
D1 = 64
M = 8
H = 256
O0 = 128
O1 = 32
NCORES = 8

WARMUP_MMS = 8

BIAS_OFF = 1024        # 28 fp32 cols, bitcast into the bf16 image
SEG0 = 1080            # w00 [0:1024] + bias [1024:1080]


def _offs(C):
    x0a = SEG0
    x1a = SEG0 + 2 * C
    w1p = SEG0 + 4 * C
    x0b = w1p + 256
    x1b = x0b + 2 * C
    wrest = x1b + 2 * C            # w01 (2048) | w1a (384) | w1o (32)
    total = wrest + 2464
    return x0a, x1a, w1p, x0b, x1b, wrest, total


def _build_bass(C, ncol):
    import concourse.bass as bass
    import concourse.tile as tile
    from concourse import bacc, mybir

    F32 = mybir.dt.float32
    BF16 = mybir.dt.bfloat16
    Relu = mybir.ActivationFunctionType.Relu
    Identity = mybir.ActivationFunctionType.Identity
    Add = mybir.AluOpType.add
    Max = mybir.AluOpType.max

    ch = C // 2
    half = ncol // 2
    X0A, X1A, W1P, X0B, X1B, WR, TOT = _offs(C)

    nc = bacc.Bacc("TRN2", target_bir_lowering=False, debug=False,
                   enable_asserts=False, num_devices=NCORES)

    inp = nc.dram_tensor("inp", [128, TOT], BF16, kind="ExternalInput").ap()
    outT = nc.dram_tensor("outT", [128, 4 * ch], F32, kind="ExternalOutput").ap()

    with tile.TileContext(nc) as tc:
        with ExitStack() as ctx:
            acts = ctx.enter_context(tc.tile_pool(name="acts", bufs=1))
            ps = ctx.enter_context(tc.tile_pool(name="ps", bufs=1, space="PSUM"))

            ins = acts.tile([128, TOT], BF16, tag="in")
            h1 = acts.tile([128, 2 * ncol], BF16, tag="h1")   # mo-major
            g1 = acts.tile([128, 2 * ncol], BF16, tag="g1")   # mo-major
            hs = acts.tile([128, ncol], BF16, tag="h")
            gs = acts.tile([128, ncol], BF16, tag="g")
            outs = acts.tile([128, 4 * ch], F32, tag="out")
            wu = acts.tile([128, 512], BF16, tag="wu")
            psall = ps.tile([128, 4096], F32, tag="ps")

            # 4 input DMA issues, consumption-ordered slices of one image
            nc.sync.dma_start(ins[:, 0:X1A], inp[:, 0:X1A])          # w00+bias+x0a
            nc.sync.dma_start(ins[:, X1A:X0B], inp[:, X1A:X0B])      # x1a+w1p
            nc.sync.dma_start(ins[:, X0B:WR], inp[:, X0B:WR])        # x0b+x1b
            nc.sync.dma_start(ins[:, WR:TOT], inp[:, WR:TOT])        # w01|w1a|w1o

            def x0c(c):    # global x0 column c -> image column
                return X0A + c if c < 2 * C else X0B + (c - 2 * C)

            def x1c(c):
                return X1A + c if c < 2 * C else X1B + (c - 2 * C)

            slot_ctr = [0]

            def alloc():
                r = slot_ctr[0] % 4
                slot_ctr[0] += 1
                return psall[:, r * 1024: r * 1024 + 1024]

            nc.vector.memset(wu[:], 0.0)
            for _ in range(WARMUP_MMS):
                pw = alloc()
                nc.tensor.matmul(pw[:, 0:512], wu[:, 0:128], wu[:],
                                 start=True, stop=True)

            bias = ins[:, BIAS_OFF:BIAS_OFF + 56].bitcast(F32)

            def bias_ap(j):
                return bias[:, j:j + 1]

            load = [0.0, 0.0]

            def evict(dst_ap, src_ap, bap, relu):
                e = 0 if load[0] + 840.0 <= load[1] + 932.0 else 1
                load[e] += 840.0 if e == 0 else 932.0
                if e == 0:
                    nc.scalar.activation(dst_ap, src_ap, Relu if relu else Identity,
                                         bias=bap)
                else:
                    if relu:
                        nc.vector.tensor_scalar(dst_ap, src_ap, bap, 0.0, Add, Max)
                    else:
                        nc.vector.tensor_scalar_add(dst_ap, src_ap, bap)

            def b2(region):
                return region[:].rearrange("p (b c) -> p b c", b=2)[:, :, 0:ch]

            def r2(dst):
                return dst.rearrange("p (b c) -> p b c", b=2)

            # ---- L1 unit (blk, mo): K=64 on a row-half, 2 chunks
            def l1_unit(blk, mo):
                rows = slice(0, 64) if blk < 4 else slice(64, 128)
                pr = alloc()
                w = ins[rows, (blk % 4) * 256 + mo * 128:
                        (blk % 4) * 256 + mo * 128 + 128]
                base = (blk % 4) * C
                for c in range(2):
                    nc.tensor.matmul(pr[:, c * 512: c * 512 + ch], w,
                                     ins[rows, x0c(base + c * ch):
                                         x0c(base + c * ch) + ch],
                                     start=True, stop=True)
                dst = h1[:, mo * ncol + blk * C: mo * ncol + (blk + 1) * C]
                evict(r2(dst), b2(pr), bias_ap(mo * 8 + blk), True)

            # ---- L3 unit (k, mo, hi): K=64 row-half, chunks 2k,2k+1
            def l3_unit(k, mo, hihalf):
                rows = slice(64, 128) if hihalf else slice(0, 64)
                pr = alloc()
                w = ins[rows, W1P + mo * 128: W1P + mo * 128 + 128]
                for j in range(2):
                    cc = k * C + j * ch
                    nc.tensor.matmul(pr[:, j * 512: j * 512 + ch], w,
                                     ins[rows, x1c(cc): x1c(cc) + ch],
                                     start=True, stop=True)
                go = mo * ncol + (half if hihalf else 0) + k * C
                evict(r2(g1[:, go: go + C]), b2(pr), bias_ap(24 + mo), True)

            for blk in (0, 1):
                for mo in range(2):
                    l1_unit(blk, mo)
                    l1_unit(blk + 4, mo)
            for k in (0, 1):
                for mo in range(2):
                    l3_unit(k, mo, False)
                    l3_unit(k, mo, True)
            for blk in (2, 3):
                for mo in range(2):
                    l1_unit(blk, mo)
                    l1_unit(blk + 4, mo)
            for k in (2, 3):
                for mo in range(2):
                    l3_unit(k, mo, False)
                    l3_unit(k, mo, True)

            # ---- L2 unit (blk): K=256, 2 accum chunks
            for blk in range(M):
                pr = alloc()
                for c in range(2):
                    for kc in range(2):
                        w = ins[:, WR + kc * 1024 + blk * 128:
                                WR + kc * 1024 + blk * 128 + 128]
                        nc.tensor.matmul(
                            pr[:, c * 512: c * 512 + ch], w,
                            h1[:, kc * ncol + blk * C + c * ch:
                               kc * ncol + blk * C + (c + 1) * ch],
                            start=(kc == 0), stop=(kc == 1))
                evict(r2(hs[:, blk * C: (blk + 1) * C]), b2(pr),
                      bias_ap(16 + blk), True)

            # ---- L4 unit (u): K=384 (3 accum), chunks 2u, 2u+1
            for u in range(M):
                pr = alloc()
                for j in range(2):
                    cc = (2 * u + j) * ch
                    for kc in range(3):
                        rhs = hs[:, cc: cc + ch] if kc == 0 else \
                            g1[:, (kc - 1) * ncol + cc: (kc - 1) * ncol + cc + ch]
                        w = ins[:, WR + 2048 + kc * 128: WR + 2048 + kc * 128 + 128]
                        nc.tensor.matmul(pr[:, j * 512: j * 512 + ch], w, rhs,
                                         start=(kc == 0), stop=(kc == 2))
                evict(r2(gs[:, 2 * u * ch: (2 * u + 2) * ch]), b2(pr),
                      bias_ap(26), True)

            # ---- L5 unit (t): col-tiled 4x32, identity+bias
            w1o = ins[:, WR + 2432: WR + 2464]
            for t in range(2):
                pr = alloc()
                for j, s in enumerate((2 * t, 2 * t + 1)):
                    for g in range(4):
                        cc = (4 * s + g) * ch
                        nc.tensor.matmul(
                            pr[32 * g: 32 * g + 32, j * 512: j * 512 + ch],
                            w1o, gs[:, cc: cc + ch],
                            start=True, stop=True, tile_position=(0, 32 * g))
                evict(r2(outs[:, 2 * t * ch: (2 * t + 2) * ch]), b2(pr),
                      bias_ap(27), False)
                nc.gpsimd.dma_start(outT[:, 2 * t * ch: (2 * t + 2) * ch],
                                    outs[:, 2 * t * ch: (2 * t + 2) * ch])

    nc.compile()
    return nc


def _route(input_val):
    a0 = np.argmax(input_val[:, D0 + D1: D0 + D1 + M], axis=1)
    a1 = np.argmax(input_val[:, D0 + D1 + M: D0 + D1 + 2 * M], axis=1)
    B = input_val.shape[0]
    nij = np.zeros((M, M), dtype=np.int64)
    np.add.at(nij, (a1, a0), 1)
    C = int(-(-nij.max() // 4) * 4)
    ncol = M * C

    key = a1 * M + a0
    order = np.argsort(key, kind="stable")
    counts = np.bincount(key, minlength=M * M)
    group_start = np.concatenate([[0], np.cumsum(counts)[:-1]])
    rank_sorted = np.arange(B) - np.repeat(group_start, counts)
    rank = np.empty(B, dtype=np.int64)
    rank[order] = rank_sorted
    assert np.all(rank < C), "capacity overflow"
    return a1, a0 * C + rank, a0, C, ncol


def _pack(inputs, X0T, X1T, C):
    import ml_dtypes
    bf = ml_dtypes.bfloat16
    X0A, X1A, W1P, X0B, X1B, WR, TOT = _offs(C)
    W00 = inputs["W00"].astype(np.float32)
    W01 = inputs["W01"].astype(np.float32)
    W1p = inputs["W1p"].astype(np.float32)
    W1a = inputs["W1a"].astype(np.float32)
    W1o = inputs["W1o"].astype(np.float32)
    b00 = inputs["b00"].astype(np.float32)
    b01 = inputs["b01"].astype(np.float32)
    b1p = inputs["b1p"].astype(np.float32)
    b1a = inputs["b1a"].astype(np.float32)
    b1o = inputs["b1o"].astype(np.float32)

    imgs = []
    for c in range(NCORES):
        img = np.zeros((128, TOT), dtype=bf)
        img[0:64, 0:1024] = W00[:4].transpose(1, 0, 2).reshape(64, 1024).astype(bf)
        img[64:128, 0:1024] = W00[4:].transpose(1, 0, 2).reshape(64, 1024).astype(bf)
        bias = np.zeros((128, 28), dtype=np.float32)
        bias[:, 0:16] = b00.reshape(M, 2, 128).transpose(2, 1, 0).reshape(128, 16)
        bias[:, 16:24] = b01.T
        bias[:, 24:26] = b1p[c].reshape(2, 128).T
        bias[:, 26] = b1a[c]
        bias[:, 27] = np.tile(b1o[c], 4)
        img[:, BIAS_OFF:BIAS_OFF + 56] = bias.view(np.uint16).view(bf)
        img[:, X0A:X0A + 2 * C] = X0T[c][:, 0:2 * C]
        img[:, X1A:X1A + 2 * C] = X1T[c][:, 0:2 * C]
        wp = W1p[c].astype(bf)
        img[0:64, W1P:W1P + 256] = wp
        img[64:128, W1P:W1P + 256] = wp
        img[:, X0B:X0B + 2 * C] = X0T[c][:, 2 * C:4 * C]
        img[:, X1B:X1B + 2 * C] = X1T[c][:, 2 * C:4 * C]
        img[:, WR:WR + 2048] = W01.reshape(M, 2, 128, O0).transpose(
            2, 1, 0, 3).reshape(128, 2048).astype(bf)
        img[:, WR + 2048:WR + 2432] = W1a[c].reshape(3, 128, O0).transpose(
            1, 0, 2).reshape(128, 384).astype(bf)
        img[:, WR + 2432:WR + 2464] = W1o[c].astype(bf)
        imgs.append(np.ascontiguousarray(img))
    return imgs


def kernel(**inputs):
    import os
    import ml_dtypes
    from concourse.bass_utils import run_bass_kernel_spmd

    bf = ml_dtypes.bfloat16
    input_val = np.asarray(inputs["input_val"], dtype=np.float32)

    core, col, a0, C, ncol = _route(input_val)
    half = ncol // 2
    ch = C // 2

    feat0 = input_val[:, :D0].astype(bf)
    feat1 = input_val[:, D0:D0 + D1].astype(bf)
    X0T = np.zeros((NCORES, 128, half), dtype=bf)
    X1T = np.zeros((NCORES, 128, half), dtype=bf)
    hi = a0 >= 4
    prow = np.where(hi, 64, 0)
    pcol = np.where(hi, col - half, col)
    for r in (0, 64):
        m = prow == r
        X0T[core[m], r:r + 64, pcol[m]] = feat0[m]
        X1T[core[m], r:r + 64, pcol[m]] = feat1[m]

    imgs = _pack(inputs, X0T, X1T, C)
    nc = _build_bass(C, ncol)

    in_maps = [dict(inp=imgs[c]) for c in range(NCORES)]
    res = run_bass_kernel_spmd(nc, in_maps, core_ids=list(range(NCORES)),
                               tmpdir=os.environ.get("BASS_TMPDIR"))
    global _LAST_RESULTS
    _LAST_RESULTS = res

    OUT = np.stack([r["outT"] for r in res.results])
    k = col // ch
    cc = col % ch
    rows = (32 * (k % 4))[:, None] + np.arange(O1)[None, :]
    cols = ((k // 4) * ch + cc)[:, None]
    out = OUT[core[:, None], rows, cols]
    return np.ascontiguousarray(out).astype(np.float32)
